# revision 16
# baseline (speedup 1.0000x reference)
"""SupCon loss (nn_ConLoss) on 8 Trainium2 NeuronCores.

Math: the reference builds logits = anchor @ contrast.T with anchor rows
being label-gathered prototypes, so logits has only N_CLASSES=100 distinct
rows.  Everything factors through P = protos @ contrast.T  [100, V*B]:

  per class c:  M[c]  = max_j P[c,j]
                E[c]  = sum_j exp((P[c,j]-M[c])/T)
                G[c]  = sum_{j: l_j==c} P[c,j]
  per column j: d[j]  = P[l_j, j]                (diagonal of the big logits)

  row i (label c=l_i):  S_i   = E[c]·exp(...) - exp(d_i/T - M[c]/T)
                        numer = G[c]/T - V·cnt[c]·M[c]/T - (d_i/T - M[c]/T)
                        mlpp  = numer/(V·cnt[c]-1) - log S_i
  loss = -mean(mlpp)

Sharding: the V*B = 8192 contrast columns are split 1024 per core (this is
simultaneously an anchor-row shard since row i pairs with column i).  Each
core computes P_shard = protos @ contrast_shard.T on the tensor engine plus
the per-class partial stats (max / exp-sum / masked sum) and the diagonal
gather (one-hot mask matmul).  The tiny [100]-sized partials are merged on
the host (the "all-reduce" of the scalar loss mean).
"""

import numpy as np

import bass_rust
import concourse.bass as bass
import concourse.mybir as mybir
import concourse.tile as tile
from concourse.vector_clock import ScopedClock
from concourse.bass_utils import run_bass_kernel_spmd

B, V, D = 4096, 2, 512
N_CLASSES = 100
TEMPERATURE = 0.07
N_CORES = 8
CPB = (V * B) // N_CORES          # contrast columns per core = 1024
KT = D // 128                     # K-tiles of 128 = 4

def _split_multi_waits(nc):
    """This walrus build rejects instructions carrying more than one sync
    wait.  Hoist extra waits onto same-engine NOPs inserted immediately
    before the instruction (waits execute in program order on the same
    sequencer, so semantics are unchanged)."""
    n = 0
    for f in nc.m.functions:
        for b in f.blocks:
            insts = b.instructions  # live list
            i = 0
            while i < len(insts):
                inst = insts[i]
                si = inst.sync_info
                waits = list(si.on_wait) if si and si.on_wait else []
                if len(waits) > 1:
                    inst.sync_info = bass_rust.SyncInfo(
                        on_wait=waits[-1:], on_update=list(si.on_update or [])
                    )
                    for w in waits[:-1]:
                        nop = mybir.InstNoOp(name=f"waitsplit-{n}", ins=[], outs=[])
                        n += 1
                        nop.engine = inst.engine
                        nop.sync_info = bass_rust.SyncInfo(on_wait=[w], on_update=[])
                        insts.insert(i, nop)
                        i += 1
                i += 1


_nc_cache = None


def _build_program():
    global _nc_cache
    if _nc_cache is not None:
        return _nc_cache

    from concourse.masks import make_identity

    f32 = mybir.dt.float32
    f32r = mybir.dt.float32r
    bf16 = mybir.dt.bfloat16
    u8 = mybir.dt.uint8
    nc = bass.Bass()
    # bf16 inputs: the matmul accumulates fp32 in PSUM; input rounding gives
    # ~1e-1 abs error on P (|P|~100), which the host combine averages down to
    # ~5e-5 relative on the scalar loss (measured) while halving the
    # DMA-bound input bytes.  ct layout: [p, n*KT*512 + a*512 + j] so each
    # 512-column half-pipeline loads with a single 512KB DMA (the HWDGE
    # fixed cost is ~625ns per dma_start -- fewer, bigger DMAs win).
    ct = nc.declare_dram_parameter("ct", [128, KT * CPB], bf16, isOutput=False)
    pt = nc.declare_dram_parameter("pt", [128, KT * N_CLASSES], bf16, isOutput=False)
    mask = nc.declare_dram_parameter("mask", [N_CLASSES, CPB], u8, isOutput=False)
    ones = nc.declare_dram_parameter("ones", [128, 1], f32r, isOutput=False)
    # single output: row 0 cols [0,CPB) = diagonal; cols [CPB,CPB+100) rows
    # 0..7 = transposed per-half stats [mx0, mx1, es0, es1, gs0, gs1, 0, 0].
    out = nc.declare_dram_parameter("out", [8, CPB + N_CLASSES], f32, isOutput=True)

    inv_t = 1.0 / TEMPERATURE
    NH = CPB // 512  # halves
    HB = KT * 512    # packed columns per half

    with tile.TileContext(nc) as tc:
        with (
            tc.tile_pool(name="singles", bufs=1) as singles,
            tc.tile_pool(name="work", bufs=1) as work,
            tc.tile_pool(name="psum", bufs=1, space="PSUM") as psum,
        ):
            # --- input DMAs: ones, pt, mask, then the two ct halves ---
            ones_t = singles.tile([128, 1], f32r)
            nc.sync.dma_start(out=ones_t, in_=ones[:, :])
            pt_t = singles.tile([128, KT * N_CLASSES], bf16)
            nc.sync.dma_start(out=pt_t, in_=pt[:, :])
            mask_t = work.tile([N_CLASSES, CPB], u8)
            nc.sync.dma_start(out=mask_t, in_=mask[:, :])
            ct_h = []
            for n in range(NH):
                t = work.tile([128, HB], bf16, name=f"cth{n}", tag=f"cth{n}")
                nc.sync.dma_start(out=t, in_=ct[:, n * HB : (n + 1) * HB])
                ct_h.append(t)

            # identity built on the otherwise-idle Pool engine
            ident_t = singles.tile([N_CLASSES, N_CLASSES], f32)
            make_identity(nc, ident_t)

            p_ps, d_ps = [], []
            for n in range(NH):
                p_ps.append(psum.tile([N_CLASSES, 512], f32, name=f"pps{n}", tag=f"pps{n}"))
                d_ps.append(psum.tile([1, 512], f32, name=f"dps{n}", tag=f"dps{n}"))
            # stats columns: 0+n mx_n, 2+n es_n, 4+n gs_n, 6:8 pad
            stats_t = work.tile([N_CLASSES, 8], f32)
            nc.vector.memset(stats_t, 0.0)
            negb = work.tile([N_CLASSES, 2], f32)
            exp_scratch = work.tile([N_CLASSES, CPB], f32)
            mp = work.tile([N_CLASSES, CPB], f32r)
            outb = work.tile([8, CPB + N_CLASSES], f32)

            # PE: all P matmuls first (so half 1 is never stuck behind
            # half 0's epilogue), then the diagonal one-hot matmuls.
            for n in range(NH):
                for a in range(KT):
                    nc.tensor.matmul(
                        p_ps[n],
                        lhsT=pt_t[:, a * N_CLASSES : (a + 1) * N_CLASSES],
                        rhs=ct_h[n][:, a * 512 : (a + 1) * 512],
                        start=(a == 0),
                        stop=(a == KT - 1),
                    )

            def mul(n):
                lo, hi = n * 512, (n + 1) * 512
                nc.vector.tensor_mul(mp[:, lo:hi], mask_t[:, lo:hi], p_ps[n])

            def rmax(n):
                nc.vector.reduce_max(
                    stats_t[:, n : n + 1], p_ps[n], axis=mybir.AxisListType.X
                )

            def rsum(n):
                lo, hi = n * 512, (n + 1) * 512
                nc.vector.reduce_sum(
                    stats_t[:, 4 + n : 5 + n], mp[:, lo:hi], axis=mybir.AxisListType.X
                )

            def dmm(n):
                lo, hi = n * 512, (n + 1) * 512
                nc.tensor.matmul(
                    d_ps[n], lhsT=ones_t[:N_CLASSES, :], rhs=mp[:, lo:hi],
                    start=True, stop=True,
                )

            def dcopy(n):
                lo, hi = n * 512, (n + 1) * 512
                nc.scalar.copy(outb[0:1, lo:hi], d_ps[n])

            def expacc(n):
                lo, hi = n * 512, (n + 1) * 512
                nc.scalar.mul(negb[:, n : n + 1], stats_t[:, n : n + 1], -inv_t)
                nc.scalar.activation(
                    out=exp_scratch[:, lo:hi],
                    in_=p_ps[n],
                    func=mybir.ActivationFunctionType.Exp,
                    bias=negb[:, n : n + 1],
                    scale=inv_t,
                    accum_out=stats_t[:, 2 + n : 3 + n],
                )

            # DVE: muls and maxes first (they gate PE dmm / ACT exp), row
            # sums last; ACT: diagonal bounces as soon as each dmm lands
            mul(0); rmax(0); dmm(0); dcopy(0); expacc(0)
            mul(1); rmax(1); dmm(1); dcopy(1); expacc(1)
            rsum(0); rsum(1)

            # transpose stats [100, 8] -> [8, 100] so its DMA is 8 big
            # descriptors instead of 100 tiny ones; diag ships separately
            # (row 0) since compute engines cannot shift partitions.
            st_ps = psum.tile([8, N_CLASSES], f32)
            nc.tensor.transpose(st_ps, stats_t, ident_t)
            nc.scalar.copy(outb[0:8, CPB : CPB + N_CLASSES], st_ps)
            nc.sync.dma_start(out=out[:, :], in_=outb)

    _split_multi_waits(nc)
    _nc_cache = nc
    return nc


def _prep_inputs(features, labels, global_protos):
    """Build the per-core input maps (shard + pack layouts on host)."""
    import ml_dtypes

    bf16 = ml_dtypes.bfloat16
    feats = np.ascontiguousarray(features, dtype=np.float32)
    protos = np.ascontiguousarray(global_protos, dtype=np.float32)
    labels = np.asarray(labels).astype(np.int64)

    # protosT [D, N] packed to [128, KT*N]: pt[p, a*N+c] = protos[c, a*128+p]
    pt = np.ascontiguousarray(
        protos.T.reshape(KT, 128, N_CLASSES).transpose(1, 0, 2).reshape(128, -1)
    ).astype(bf16)

    in_maps = []
    bpc = B // (N_CORES // V)  # batch rows per core slab = 1024
    for k in range(N_CORES):
        b0 = bpc * (k % (N_CORES // V))
        v = k // (N_CORES // V)
        slab = feats[b0 : b0 + bpc, v, :]  # [1024, 512]
        lab = labels[b0 : b0 + bpc]
        # contrastT packed [p, n*KT*512 + a*512 + j] (n-major halves)
        ct = np.ascontiguousarray(
            slab.T.reshape(KT, 128, CPB // 512, 512)
            .transpose(1, 2, 0, 3)
            .reshape(128, -1)
        ).astype(bf16)
        msk = (lab[None, :] == np.arange(N_CLASSES)[:, None]).astype(np.uint8)
        in_maps.append(
            {
                "ct": ct,
                "pt": pt,
                "mask": np.ascontiguousarray(msk),
                "ones": np.ones((128, 1), dtype=np.float32),
            }
        )
    return in_maps, labels


def _combine(results, labels):
    """Merge per-core/per-half partials into the scalar loss (float64)."""
    T = TEMPERATURE
    # out: row 0 cols [0,CPB) diag; cols [CPB,CPB+100) rows 0..5 are
    # [mx0, mx1, es0, es1, gs0, gs1]
    st = [r["out"][:, CPB : CPB + N_CLASSES] for r in results]
    mx_a = np.concatenate([s[0:2] for s in st]).astype(np.float64)   # [16, 100]
    es_a = np.concatenate([s[2:4] for s in st]).astype(np.float64)
    gs_a = np.concatenate([s[4:6] for s in st]).astype(np.float64)
    d = np.concatenate([r["out"][0, :CPB] for r in results]).astype(np.float64)

    m = mx_a.max(axis=0)                                         # [100]
    E = (es_a * np.exp((mx_a - m[None, :]) / T)).sum(axis=0)     # [100]
    G = gs_a.sum(axis=0)                                         # [100]
    cnt = np.bincount(labels, minlength=N_CLASSES).astype(np.float64)

    lfull = np.tile(labels, V)                                   # [8192]
    mT = m[lfull] / T
    dT = d / T
    S = E[lfull] - np.exp(np.minimum(dT - mT, 0.0))
    S = np.maximum(S, 1e-300)
    npos = V * cnt[lfull] - 1.0
    numer = G[lfull] / T - V * cnt[lfull] * mT - (dT - mT)
    mlpp = numer / npos - np.log(S)
    return np.float32(-np.mean(mlpp))


def run(features, labels, global_protos, trace=False):
    nc = _build_program()
    in_maps, labels64 = _prep_inputs(features, labels, global_protos)
    res = run_bass_kernel_spmd(nc, in_maps, list(range(N_CORES)), trace=trace)
    loss = _combine(res.results, labels64)
    return loss, res


def kernel(features, labels, global_protos):
    loss, _ = run(features, labels, global_protos)
    return np.array(loss, dtype=np.float32)


# revision 17
# speedup vs baseline: 1.0090x; 1.0090x over previous
"""SupCon loss (nn_ConLoss) on 8 Trainium2 NeuronCores.

Math: the reference builds logits = anchor @ contrast.T with anchor rows
being label-gathered prototypes, so logits has only N_CLASSES=100 distinct
rows.  Everything factors through P = protos @ contrast.T  [100, V*B]:

  per class c:  M[c]  = max_j P[c,j]
                E[c]  = sum_j exp((P[c,j]-M[c])/T)
                G[c]  = sum_{j: l_j==c} P[c,j]
  per column j: d[j]  = P[l_j, j]                (diagonal of the big logits)

  row i (label c=l_i):  S_i   = E[c]·exp(...) - exp(d_i/T - M[c]/T)
                        numer = G[c]/T - V·cnt[c]·M[c]/T - (d_i/T - M[c]/T)
                        mlpp  = numer/(V·cnt[c]-1) - log S_i
  loss = -mean(mlpp)

Sharding: the V*B = 8192 contrast columns are split 1024 per core (this is
simultaneously an anchor-row shard since row i pairs with column i).  Each
core computes P_shard = protos @ contrast_shard.T on the tensor engine plus
the per-class partial stats (max / exp-sum / masked sum) and the diagonal
gather (one-hot mask matmul).  The tiny [100]-sized partials are merged on
the host (the "all-reduce" of the scalar loss mean).
"""

import numpy as np

import bass_rust
import concourse.bass as bass
import concourse.mybir as mybir
import concourse.tile as tile
from concourse.vector_clock import ScopedClock
from concourse.bass_utils import run_bass_kernel_spmd

B, V, D = 4096, 2, 512
N_CLASSES = 100
TEMPERATURE = 0.07
N_CORES = 8
CPB = (V * B) // N_CORES          # contrast columns per core = 1024
KT = D // 128                     # K-tiles of 128 = 4

def _split_multi_waits(nc):
    """This walrus build rejects instructions carrying more than one sync
    wait.  Hoist extra waits onto same-engine NOPs inserted immediately
    before the instruction (waits execute in program order on the same
    sequencer, so semantics are unchanged)."""
    n = 0
    for f in nc.m.functions:
        for b in f.blocks:
            insts = b.instructions  # live list
            i = 0
            while i < len(insts):
                inst = insts[i]
                si = inst.sync_info
                waits = list(si.on_wait) if si and si.on_wait else []
                if len(waits) > 1:
                    inst.sync_info = bass_rust.SyncInfo(
                        on_wait=waits[-1:], on_update=list(si.on_update or [])
                    )
                    for w in waits[:-1]:
                        nop = mybir.InstNoOp(name=f"waitsplit-{n}", ins=[], outs=[])
                        n += 1
                        nop.engine = inst.engine
                        nop.sync_info = bass_rust.SyncInfo(on_wait=[w], on_update=[])
                        insts.insert(i, nop)
                        i += 1
                i += 1


_nc_cache = None


def _build_program():
    global _nc_cache
    if _nc_cache is not None:
        return _nc_cache

    from concourse.masks import make_identity

    f32 = mybir.dt.float32
    f32r = mybir.dt.float32r
    bf16 = mybir.dt.bfloat16
    u8 = mybir.dt.uint8
    nc = bass.Bass()
    # bf16 inputs: the matmul accumulates fp32 in PSUM; input rounding gives
    # ~1e-1 abs error on P (|P|~100), which the host combine averages down to
    # ~5e-5 relative on the scalar loss (measured) while halving the
    # DMA-bound input bytes.  ct layout: [p, n*KT*512 + a*512 + j] so each
    # 512-column half-pipeline loads with a single 512KB DMA (the HWDGE
    # fixed cost is ~625ns per dma_start -- fewer, bigger DMAs win).
    # ctp cols: [0,400) protosT | [400,2448) ct half 0 | [2448,4496) half 1;
    # loaded as two DMAs (pt+half0, half1) into two tiles so half-0 matmuls
    # start while half 1 is still in flight.
    PTW = KT * N_CLASSES
    ctp = nc.declare_dram_parameter("ctp", [128, PTW + KT * CPB], bf16, isOutput=False)
    mask = nc.declare_dram_parameter("mask", [N_CLASSES, CPB], u8, isOutput=False)
    # single output: row 0 cols [0,CPB) = diagonal; cols [CPB,CPB+100) rows
    # 0..7 = transposed per-half stats [mx0, mx1, es0, es1, gs0, gs1, 0, 0].
    out = nc.declare_dram_parameter("out", [8, CPB + N_CLASSES], f32, isOutput=True)

    inv_t = 1.0 / TEMPERATURE
    NH = CPB // 512  # halves
    HB = KT * 512    # packed columns per half

    with tile.TileContext(nc) as tc:
        with (
            tc.tile_pool(name="singles", bufs=1) as singles,
            tc.tile_pool(name="work", bufs=1) as work,
            tc.tile_pool(name="psum", bufs=1, space="PSUM") as psum,
        ):
            # --- input DMAs: (pt + ct half 0), mask, ct half 1 ---
            a_t = work.tile([128, PTW + HB], bf16, name="a_t")
            nc.sync.dma_start(out=a_t, in_=ctp[:, 0 : PTW + HB])
            mask_t = work.tile([N_CLASSES, CPB], u8)
            nc.sync.dma_start(out=mask_t, in_=mask[:, :])
            b_t = work.tile([128, HB], bf16, name="b_t")
            nc.sync.dma_start(out=b_t, in_=ctp[:, PTW + HB : PTW + 2 * HB])
            pt_t = a_t[:, 0:PTW]
            ct_h = [a_t[:, PTW : PTW + HB], b_t[:, :]]

            # f32r ones: memset cannot write f32r, but a DVE copy can convert
            ones_f = singles.tile([128, 1], f32)
            nc.vector.memset(ones_f, 1.0)
            ones_t = singles.tile([128, 1], f32r)
            nc.vector.tensor_copy(ones_t, ones_f)

            # identity built on the otherwise-idle Pool engine
            ident_t = singles.tile([N_CLASSES, N_CLASSES], f32)
            make_identity(nc, ident_t)

            p_ps, d_ps = [], []
            for n in range(NH):
                p_ps.append(psum.tile([N_CLASSES, 512], f32, name=f"pps{n}", tag=f"pps{n}"))
                d_ps.append(psum.tile([1, 512], f32, name=f"dps{n}", tag=f"dps{n}"))
            # stats columns: 0+n mx_n, 2+n es_n, 4+n gs_n, 6:8 pad
            stats_t = work.tile([N_CLASSES, 8], f32)
            nc.vector.memset(stats_t, 0.0)
            negb = work.tile([N_CLASSES, 2], f32)
            exp_scratch = work.tile([N_CLASSES, CPB], f32)
            mp = work.tile([N_CLASSES, CPB], f32r)
            outb = work.tile([8, CPB + N_CLASSES], f32)

            # PE: all P matmuls first (so half 1 is never stuck behind
            # half 0's epilogue), then the diagonal one-hot matmuls.
            for n in range(NH):
                for a in range(KT):
                    nc.tensor.matmul(
                        p_ps[n],
                        lhsT=pt_t[:, a * N_CLASSES : (a + 1) * N_CLASSES],
                        rhs=ct_h[n][:, a * 512 : (a + 1) * 512],
                        start=(a == 0),
                        stop=(a == KT - 1),
                    )

            def mul(n):
                lo, hi = n * 512, (n + 1) * 512
                nc.vector.tensor_mul(mp[:, lo:hi], mask_t[:, lo:hi], p_ps[n])

            def rmax(n):
                nc.vector.reduce_max(
                    stats_t[:, n : n + 1], p_ps[n], axis=mybir.AxisListType.X
                )

            def rsum(n):
                lo, hi = n * 512, (n + 1) * 512
                nc.vector.reduce_sum(
                    stats_t[:, 4 + n : 5 + n], mp[:, lo:hi], axis=mybir.AxisListType.X
                )

            def dmm(n):
                lo, hi = n * 512, (n + 1) * 512
                nc.tensor.matmul(
                    d_ps[n], lhsT=ones_t[:N_CLASSES, :], rhs=mp[:, lo:hi],
                    start=True, stop=True,
                )

            def dcopy(n):
                lo, hi = n * 512, (n + 1) * 512
                nc.scalar.copy(outb[0:1, lo:hi], d_ps[n])

            def expacc(n):
                lo, hi = n * 512, (n + 1) * 512
                nc.scalar.mul(negb[:, n : n + 1], stats_t[:, n : n + 1], -inv_t)
                nc.scalar.activation(
                    out=exp_scratch[:, lo:hi],
                    in_=p_ps[n],
                    func=mybir.ActivationFunctionType.Exp,
                    bias=negb[:, n : n + 1],
                    scale=inv_t,
                    accum_out=stats_t[:, 2 + n : 3 + n],
                )

            # DVE: muls and maxes first (they gate PE dmm / ACT exp), row
            # sums last; ACT: diagonal bounces as soon as each dmm lands
            mul(0); rmax(0); dmm(0); expacc(0); dcopy(0)
            mul(1); rmax(1); dmm(1); expacc(1); dcopy(1)
            rsum(0); rsum(1)

            # transpose stats [100, 8] -> [8, 100] so its DMA is 8 big
            # descriptors instead of 100 tiny ones; diag ships separately
            # (row 0) since compute engines cannot shift partitions.
            st_ps = psum.tile([8, N_CLASSES], f32)
            nc.tensor.transpose(st_ps, stats_t, ident_t)
            nc.scalar.copy(outb[0:8, CPB : CPB + N_CLASSES], st_ps)
            nc.sync.dma_start(out=out[:, :], in_=outb)

    _split_multi_waits(nc)
    _nc_cache = nc
    return nc


def _prep_inputs(features, labels, global_protos):
    """Build the per-core input maps (shard + pack layouts on host)."""
    import ml_dtypes

    bf16 = ml_dtypes.bfloat16
    feats = np.ascontiguousarray(features, dtype=np.float32)
    protos = np.ascontiguousarray(global_protos, dtype=np.float32)
    labels = np.asarray(labels).astype(np.int64)

    # protosT [D, N] packed to [128, KT*N]: pt[p, a*N+c] = protos[c, a*128+p]
    pt = (
        protos.T.reshape(KT, 128, N_CLASSES).transpose(1, 0, 2).reshape(128, -1)
    ).astype(bf16)

    in_maps = []
    bpc = B // (N_CORES // V)  # batch rows per core slab = 1024
    for k in range(N_CORES):
        b0 = bpc * (k % (N_CORES // V))
        v = k // (N_CORES // V)
        slab = feats[b0 : b0 + bpc, v, :]  # [1024, 512]
        lab = labels[b0 : b0 + bpc]
        # contrastT packed [p, n*KT*512 + a*512 + j] (n-major halves),
        # prefixed with protosT so pt + half 0 load as one DMA
        ct = (
            slab.T.reshape(KT, 128, CPB // 512, 512)
            .transpose(1, 2, 0, 3)
            .reshape(128, -1)
        ).astype(bf16)
        ctp = np.ascontiguousarray(np.concatenate([pt, ct], axis=1))
        msk = (lab[None, :] == np.arange(N_CLASSES)[:, None]).astype(np.uint8)
        in_maps.append({"ctp": ctp, "mask": np.ascontiguousarray(msk)})
    return in_maps, labels


def _combine(results, labels):
    """Merge per-core/per-half partials into the scalar loss (float64)."""
    T = TEMPERATURE
    # out: row 0 cols [0,CPB) diag; cols [CPB,CPB+100) rows 0..5 are
    # [mx0, mx1, es0, es1, gs0, gs1]
    st = [r["out"][:, CPB : CPB + N_CLASSES] for r in results]
    mx_a = np.concatenate([s[0:2] for s in st]).astype(np.float64)   # [16, 100]
    es_a = np.concatenate([s[2:4] for s in st]).astype(np.float64)
    gs_a = np.concatenate([s[4:6] for s in st]).astype(np.float64)
    d = np.concatenate([r["out"][0, :CPB] for r in results]).astype(np.float64)

    m = mx_a.max(axis=0)                                         # [100]
    E = (es_a * np.exp((mx_a - m[None, :]) / T)).sum(axis=0)     # [100]
    G = gs_a.sum(axis=0)                                         # [100]
    cnt = np.bincount(labels, minlength=N_CLASSES).astype(np.float64)

    lfull = np.tile(labels, V)                                   # [8192]
    mT = m[lfull] / T
    dT = d / T
    S = E[lfull] - np.exp(np.minimum(dT - mT, 0.0))
    S = np.maximum(S, 1e-300)
    npos = V * cnt[lfull] - 1.0
    numer = G[lfull] / T - V * cnt[lfull] * mT - (dT - mT)
    mlpp = numer / npos - np.log(S)
    return np.float32(-np.mean(mlpp))


def run(features, labels, global_protos, trace=False):
    nc = _build_program()
    in_maps, labels64 = _prep_inputs(features, labels, global_protos)
    res = run_bass_kernel_spmd(nc, in_maps, list(range(N_CORES)), trace=trace)
    loss = _combine(res.results, labels64)
    return loss, res


def kernel(features, labels, global_protos):
    loss, _ = run(features, labels, global_protos)
    return np.array(loss, dtype=np.float32)


# revision 18
# speedup vs baseline: 1.0360x; 1.0267x over previous
"""SupCon loss (nn_ConLoss) on 8 Trainium2 NeuronCores.

Math: the reference builds logits = anchor @ contrast.T with anchor rows
being label-gathered prototypes, so logits has only N_CLASSES=100 distinct
rows.  Everything factors through P = protos @ contrast.T  [100, V*B]:

  per class c:  M[c]  = max_j P[c,j]
                E[c]  = sum_j exp((P[c,j]-M[c])/T)
                G[c]  = sum_{j: l_j==c} P[c,j]
  per column j: d[j]  = P[l_j, j]                (diagonal of the big logits)

  row i (label c=l_i):  S_i   = E[c]·exp(...) - exp(d_i/T - M[c]/T)
                        numer = G[c]/T - V·cnt[c]·M[c]/T - (d_i/T - M[c]/T)
                        mlpp  = numer/(V·cnt[c]-1) - log S_i
  loss = -mean(mlpp)

Sharding: the V*B = 8192 contrast columns are split 1024 per core (this is
simultaneously an anchor-row shard since row i pairs with column i).  Each
core computes P_shard = protos @ contrast_shard.T on the tensor engine plus
the per-class partial stats (max / exp-sum / masked sum) and the diagonal
gather (one-hot mask matmul).  The tiny [100]-sized partials are merged on
the host (the "all-reduce" of the scalar loss mean).
"""

import numpy as np

import bass_rust
import concourse.bass as bass
import concourse.mybir as mybir
import concourse.tile as tile
from concourse.vector_clock import ScopedClock
from concourse.bass_utils import run_bass_kernel_spmd

B, V, D = 4096, 2, 512
N_CLASSES = 100
TEMPERATURE = 0.07
N_CORES = 8
CPB = (V * B) // N_CORES          # contrast columns per core = 1024
KT = D // 128                     # K-tiles of 128 = 4

def _split_multi_waits(nc):
    """This walrus build rejects instructions carrying more than one sync
    wait.  Hoist extra waits onto same-engine NOPs inserted immediately
    before the instruction (waits execute in program order on the same
    sequencer, so semantics are unchanged)."""
    n = 0
    for f in nc.m.functions:
        for b in f.blocks:
            insts = b.instructions  # live list
            i = 0
            while i < len(insts):
                inst = insts[i]
                si = inst.sync_info
                waits = list(si.on_wait) if si and si.on_wait else []
                if len(waits) > 1:
                    inst.sync_info = bass_rust.SyncInfo(
                        on_wait=waits[-1:], on_update=list(si.on_update or [])
                    )
                    for w in waits[:-1]:
                        nop = mybir.InstNoOp(name=f"waitsplit-{n}", ins=[], outs=[])
                        n += 1
                        nop.engine = inst.engine
                        nop.sync_info = bass_rust.SyncInfo(on_wait=[w], on_update=[])
                        insts.insert(i, nop)
                        i += 1
                i += 1


_nc_cache = None


def _build_program():
    global _nc_cache
    if _nc_cache is not None:
        return _nc_cache

    from concourse.masks import make_identity

    f32 = mybir.dt.float32
    f32r = mybir.dt.float32r
    bf16 = mybir.dt.bfloat16
    u8 = mybir.dt.uint8
    nc = bass.Bass()
    # bf16 inputs: the matmul accumulates fp32 in PSUM; input rounding gives
    # ~1e-1 abs error on P (|P|~100), which the host combine averages down to
    # ~5e-5 relative on the scalar loss (measured) while halving the
    # DMA-bound input bytes.  ct layout: [p, n*KT*512 + a*512 + j] so each
    # 512-column half-pipeline loads with a single 512KB DMA (the HWDGE
    # fixed cost is ~625ns per dma_start -- fewer, bigger DMAs win).
    # ctp cols: [0,400) protosT | [400,2448) ct half 0 | [2448,4496) half 1;
    # loaded as two DMAs (pt+half0, half1) into two tiles so half-0 matmuls
    # start while half 1 is still in flight.
    PTW = KT * N_CLASSES
    ctp = nc.declare_dram_parameter("ctp", [128, PTW + KT * CPB], bf16, isOutput=False)
    mask = nc.declare_dram_parameter("mask", [N_CLASSES, CPB], u8, isOutput=False)
    # single output: row 0 cols [0,CPB) = diagonal; cols [CPB,CPB+100) rows
    # 0..7 = transposed per-half stats [mx0, mx1, es0, es1, gs0, gs1, 0, 0].
    out = nc.declare_dram_parameter("out", [8, CPB + N_CLASSES], f32, isOutput=True)

    inv_t = 1.0 / TEMPERATURE
    NH = CPB // 512  # halves
    HB = KT * 512    # packed columns per half

    with tile.TileContext(nc) as tc:
        with (
            tc.tile_pool(name="singles", bufs=1) as singles,
            tc.tile_pool(name="work", bufs=1) as work,
            tc.tile_pool(name="psum", bufs=1, space="PSUM") as psum,
        ):
            # --- input DMAs: (pt + ct half 0), mask, ct half 1 ---
            a_t = work.tile([128, PTW + HB], bf16, name="a_t")
            nc.sync.dma_start(out=a_t, in_=ctp[:, 0 : PTW + HB])
            mask_t = work.tile([N_CLASSES, CPB], u8)
            nc.sync.dma_start(out=mask_t, in_=mask[:, :])
            b_t = work.tile([128, HB], bf16, name="b_t")
            nc.sync.dma_start(out=b_t, in_=ctp[:, PTW + HB : PTW + 2 * HB])
            pt_t = a_t[:, 0:PTW]
            ct_h = [a_t[:, PTW : PTW + HB], b_t[:, :]]

            # f32r ones: memset cannot write f32r, but a DVE copy can convert
            ones_f = singles.tile([128, 1], f32)
            nc.vector.memset(ones_f, 1.0)
            ones_t = singles.tile([128, 1], f32r)
            nc.vector.tensor_copy(ones_t, ones_f)

            # identity built on the otherwise-idle Pool engine
            ident_t = singles.tile([N_CLASSES, N_CLASSES], f32)
            make_identity(nc, ident_t)

            p_ps, d_ps = [], []
            for n in range(NH):
                p_ps.append(psum.tile([N_CLASSES, 512], f32, name=f"pps{n}", tag=f"pps{n}"))
                d_ps.append(psum.tile([1, 512], f32, name=f"dps{n}", tag=f"dps{n}"))
            # stats columns: 0+n mx_n, 2+n es_n, 4+n gs_n, 6:8 pad
            stats_t = work.tile([N_CLASSES, 8], f32)
            nc.vector.memset(stats_t, 0.0)
            negb = work.tile([N_CLASSES, 2], f32)
            exp_scratch = work.tile([N_CLASSES, CPB], f32)
            mp = work.tile([N_CLASSES, CPB], f32r)
            outb = work.tile([8, CPB + N_CLASSES], f32)

            # PE warm-up primers: the HAM clock gate halves PE throughput
            # until ~3.4us of sustained activity.  Chew on the identity tile
            # (ready early, no DMA dep) so the real matmuls start at full
            # rate the moment ct lands.
            warm_ps = psum.tile([1, 64], f32, name="warm_ps")
            for _ in range(24):
                nc.tensor.matmul(
                    warm_ps, lhsT=ident_t[:, 0:1], rhs=ident_t[:, 0:64],
                    start=True, stop=True,
                )

            # PE: all P matmuls first (so half 1 is never stuck behind
            # half 0's epilogue), then the diagonal one-hot matmuls.
            for n in range(NH):
                for a in range(KT):
                    nc.tensor.matmul(
                        p_ps[n],
                        lhsT=pt_t[:, a * N_CLASSES : (a + 1) * N_CLASSES],
                        rhs=ct_h[n][:, a * 512 : (a + 1) * 512],
                        start=(a == 0),
                        stop=(a == KT - 1),
                    )

            def mul(n):
                lo, hi = n * 512, (n + 1) * 512
                nc.vector.tensor_mul(mp[:, lo:hi], mask_t[:, lo:hi], p_ps[n])

            def rmax(n):
                nc.vector.reduce_max(
                    stats_t[:, n : n + 1], p_ps[n], axis=mybir.AxisListType.X
                )

            def rsum(n):
                lo, hi = n * 512, (n + 1) * 512
                nc.vector.reduce_sum(
                    stats_t[:, 4 + n : 5 + n], mp[:, lo:hi], axis=mybir.AxisListType.X
                )

            def dmm(n):
                lo, hi = n * 512, (n + 1) * 512
                nc.tensor.matmul(
                    d_ps[n], lhsT=ones_t[:N_CLASSES, :], rhs=mp[:, lo:hi],
                    start=True, stop=True,
                )

            def dcopy(n):
                lo, hi = n * 512, (n + 1) * 512
                nc.scalar.copy(outb[0:1, lo:hi], d_ps[n])

            def expacc(n):
                lo, hi = n * 512, (n + 1) * 512
                nc.scalar.mul(negb[:, n : n + 1], stats_t[:, n : n + 1], -inv_t)
                nc.scalar.activation(
                    out=exp_scratch[:, lo:hi],
                    in_=p_ps[n],
                    func=mybir.ActivationFunctionType.Exp,
                    bias=negb[:, n : n + 1],
                    scale=inv_t,
                    accum_out=stats_t[:, 2 + n : 3 + n],
                )

            # DVE: muls and maxes first (they gate PE dmm / ACT exp), row
            # sums last; ACT: diagonal bounces as soon as each dmm lands
            mul(0); rmax(0); dmm(0); expacc(0); dcopy(0)
            mul(1); rmax(1); dmm(1); expacc(1); dcopy(1)
            rsum(0); rsum(1)

            # transpose stats [100, 8] -> [8, 100] so its DMA is 8 big
            # descriptors instead of 100 tiny ones; diag ships separately
            # (row 0) since compute engines cannot shift partitions.
            st_ps = psum.tile([8, N_CLASSES], f32)
            nc.tensor.transpose(st_ps, stats_t, ident_t)
            nc.scalar.copy(outb[0:8, CPB : CPB + N_CLASSES], st_ps)
            nc.sync.dma_start(out=out[:, :], in_=outb)

    _split_multi_waits(nc)
    _nc_cache = nc
    return nc


def _prep_inputs(features, labels, global_protos):
    """Build the per-core input maps (shard + pack layouts on host)."""
    import ml_dtypes

    bf16 = ml_dtypes.bfloat16
    feats = np.ascontiguousarray(features, dtype=np.float32)
    protos = np.ascontiguousarray(global_protos, dtype=np.float32)
    labels = np.asarray(labels).astype(np.int64)

    # protosT [D, N] packed to [128, KT*N]: pt[p, a*N+c] = protos[c, a*128+p]
    pt = (
        protos.T.reshape(KT, 128, N_CLASSES).transpose(1, 0, 2).reshape(128, -1)
    ).astype(bf16)

    in_maps = []
    bpc = B // (N_CORES // V)  # batch rows per core slab = 1024
    for k in range(N_CORES):
        b0 = bpc * (k % (N_CORES // V))
        v = k // (N_CORES // V)
        slab = feats[b0 : b0 + bpc, v, :]  # [1024, 512]
        lab = labels[b0 : b0 + bpc]
        # contrastT packed [p, n*KT*512 + a*512 + j] (n-major halves),
        # prefixed with protosT so pt + half 0 load as one DMA
        ct = (
            slab.T.reshape(KT, 128, CPB // 512, 512)
            .transpose(1, 2, 0, 3)
            .reshape(128, -1)
        ).astype(bf16)
        ctp = np.ascontiguousarray(np.concatenate([pt, ct], axis=1))
        msk = (lab[None, :] == np.arange(N_CLASSES)[:, None]).astype(np.uint8)
        in_maps.append({"ctp": ctp, "mask": np.ascontiguousarray(msk)})
    return in_maps, labels


def _combine(results, labels):
    """Merge per-core/per-half partials into the scalar loss (float64)."""
    T = TEMPERATURE
    # out: row 0 cols [0,CPB) diag; cols [CPB,CPB+100) rows 0..5 are
    # [mx0, mx1, es0, es1, gs0, gs1]
    st = [r["out"][:, CPB : CPB + N_CLASSES] for r in results]
    mx_a = np.concatenate([s[0:2] for s in st]).astype(np.float64)   # [16, 100]
    es_a = np.concatenate([s[2:4] for s in st]).astype(np.float64)
    gs_a = np.concatenate([s[4:6] for s in st]).astype(np.float64)
    d = np.concatenate([r["out"][0, :CPB] for r in results]).astype(np.float64)

    m = mx_a.max(axis=0)                                         # [100]
    E = (es_a * np.exp((mx_a - m[None, :]) / T)).sum(axis=0)     # [100]
    G = gs_a.sum(axis=0)                                         # [100]
    cnt = np.bincount(labels, minlength=N_CLASSES).astype(np.float64)

    lfull = np.tile(labels, V)                                   # [8192]
    mT = m[lfull] / T
    dT = d / T
    S = E[lfull] - np.exp(np.minimum(dT - mT, 0.0))
    S = np.maximum(S, 1e-300)
    npos = V * cnt[lfull] - 1.0
    numer = G[lfull] / T - V * cnt[lfull] * mT - (dT - mT)
    mlpp = numer / npos - np.log(S)
    return np.float32(-np.mean(mlpp))


def run(features, labels, global_protos, trace=False):
    nc = _build_program()
    in_maps, labels64 = _prep_inputs(features, labels, global_protos)
    res = run_bass_kernel_spmd(nc, in_maps, list(range(N_CORES)), trace=trace)
    loss = _combine(res.results, labels64)
    return loss, res


def kernel(features, labels, global_protos):
    loss, _ = run(features, labels, global_protos)
    return np.array(loss, dtype=np.float32)
